# revision 13
# baseline (speedup 1.0000x reference)
"""Bahdanau attention kernel for Trainium2 (8 NeuronCores) — fp8 DoubleRow.

Reference computation (B=32, S=2048, D=1024):
    x      = concat([broadcast(hidden), encoder_outputs], -1)   # [B,S,2D]
    energy = tanh(x @ W + b)                                    # [B,S,D]
    scores = energy . v                                         # [B,S]
    attn   = softmax(mask(scores, src_len))                     # [B,1,S]

Key transformations:
  * x @ W = encoder_outputs @ W[D:] + (hidden @ W[:D]); the hidden part is a
    tiny per-batch bias computed on the host and folded into the tanh bias.
  * rows with s >= src_len[b] are masked out of the softmax, so they are
    never computed: the host packs only the valid rows (padded to SUB=256
    per batch), load-balances batches across the 8 cores, and the device
    runs a dense kernel on the packed rows.
  * the big matmul runs in fp8(e4m3) with the PE's DoubleRow perf mode:
    256-deep contraction per pass, 2x bf16/fp32r throughput.  E is scaled
    by 16 and W by 256 on the host to center the e4m3 dynamic range; the
    exact 1/4096 rescale rides the tanh's activation scale.
  * the v-dot runs off the PE: ACT emits fp16 tanh tiles, the DVE
    accumulates v_m * th_m across the 8 D-chunks (2x fp16 mode), and a
    single fp16 ones-matmul reduces partitions into the [1, R] scores.
  * masking + softmax run on the host (cheap, O(B*S)).
"""

import os
import sys

import numpy as np

for _p in ("/root/.axon_site/_ro/trn_rl_repo", "/opt/trn_rl_repo"):
    if os.path.isdir(_p) and _p not in sys.path:
        sys.path.append(_p)

B, S, D = 32, 2048, 1024
N_CORES = 8
SUB = 256  # per-batch row padding == tanh bias subtile width
RB = 512  # main matmul row tile (PSUM free dim)
KS = D // 128  # 128-deep K chunks (8)
KP = KS // 2  # DoubleRow K pairs (4)
MO = D // 128  # D_out chunks (8)
E_SCALE = 16.0  # host scale on E before e4m3 quantization
W_SCALE = 256.0  # host scale on W before e4m3 quantization

_NC_CACHE = {}


def _ensure_trace_support():
    """Make trace=True / BASS_TRACE=1 runs survive on images where
    ``antenv.axon_hooks`` is absent (the boot shim degrades silently but
    ``bass_utils`` imports it unconditionally) and where artifact uploads
    to remote storage are unavailable.  No-ops when everything exists."""
    import types

    try:
        import antenv

        try:
            import antenv.axon_hooks  # noqa: F401
        except ImportError:
            mod = types.ModuleType("antenv.axon_hooks")
            state = {"hook": None}
            mod.set_axon_ntff_profile_hook = lambda h: state.__setitem__("hook", h)
            mod.get_axon_ntff_profile_hook = lambda: state["hook"]
            sys.modules["antenv.axon_hooks"] = mod
            antenv.axon_hooks = mod
            try:
                from trn_agent_boot.trn_boot import _ntff_profile_via_ctypes

                so = "/opt/axon/libaxon_pjrt.so"
                if os.path.exists(so):
                    mod.set_axon_ntff_profile_hook(_ntff_profile_via_ctypes(so))
            except Exception:
                pass
    except Exception:
        pass
    try:
        import concourse.bass_utils as bu

        orig = bu.upload_artifacts
        if not getattr(orig, "_safe_wrapped", False):

            def _safe_upload(tmpdir, _orig=orig):
                try:
                    return _orig(tmpdir)
                except Exception:
                    return f"local://{tmpdir}"

            _safe_upload._safe_wrapped = True
            bu.upload_artifacts = _safe_upload
    except Exception:
        pass


def _row_tiles(R):
    """Row-tile sizes covering R rows: 512-tiles plus an optional 256 tail."""
    tiles = [RB] * (R // RB)
    if R % RB:
        assert R % RB == 256
        tiles.append(256)
    return tiles


def _build_bass(R):
    """Build the per-core SPMD program for R packed rows (R % 256 == 0)."""
    import concourse.bass as bass  # noqa: F401
    import concourse.tile as tile
    from concourse import bacc, mybir

    f32 = mybir.dt.float32
    f16 = mybir.dt.float16
    f8 = mybir.dt.float8e4
    DR = mybir.MatmulPerfMode.DoubleRow
    n_sub = R // SUB
    tiles = _row_tiles(R)
    inv_scale = 1.0 / (E_SCALE * W_SCALE)

    nc = bacc.Bacc()
    et_d = nc.dram_tensor("et", [D, R], f8, kind="ExternalInput")
    w_d = nc.dram_tensor("wt", [D, D], f8, kind="ExternalInput")
    v_d = nc.dram_tensor("vt", [D], f32, kind="ExternalInput")
    b_d = nc.dram_tensor("bt", [D, n_sub], f32, kind="ExternalInput")
    out_d = nc.dram_tensor("scores", [1, R], f32, kind="ExternalOutput")

    et_ap = et_d[:, :].rearrange("(ks p) r -> p ks r", p=128)
    w_ap = w_d[:, :].rearrange("(ks p) j -> p ks j", p=128)
    v_ap = v_d[:].rearrange("(mo p) -> p mo", p=128)
    b_ap = b_d[:, :].rearrange("(mo p) s -> p mo s", p=128)

    with tile.TileContext(nc) as tc:
        with (
            tc.tile_pool(name="singles", bufs=1) as singles,
            tc.tile_pool(name="warm", bufs=1) as warm,
            tc.tile_pool(name="et0", bufs=1) as et0_pool,
            tc.tile_pool(name="et", bufs=12) as et_pool,
            tc.tile_pool(name="tanh", bufs=6) as tanh_pool,
            tc.tile_pool(name="acc", bufs=10) as acc_pool,
            tc.tile_pool(name="sc", bufs=2) as sc_pool,
            tc.tile_pool(name="psum_e", bufs=6, space="PSUM") as psum_e,
            tc.tile_pool(name="psum_s", bufs=2, space="PSUM") as psum_s,
        ):
            # --- warmup: keep PE busy + load the ACT tanh table while the
            # first real DMAs are in flight (HAM un-throttles after ~3.4us
            # of PE activity; the ACT table load costs ~2.7us once).
            wact = warm.tile([128, 2], f32)
            nc.vector.memset(wact[:], 0.0)
            nc.scalar.activation(
                out=wact[:, 1:2],
                in_=wact[:, 0:1],
                func=mybir.ActivationFunctionType.Tanh,
                bias=0.0,
                scale=1.0,
            )
            ones_sb = singles.tile([128, 1], f16)
            nc.vector.memset(ones_sb[:], 1.0)

            # --- E^T row-block loads: two half-K DMAs per block so
            # dependencies unblock earlier.  Issued with a prefetch depth
            # of 6 blocks; block 0 is loaded per-K-pair (below) so the
            # pipeline can start as early as possible.
            row_offs = []
            r0 = 0
            for NT in tiles:
                row_offs.append(r0)
                r0 += NT

            et_tiles = {}

            def issue_et(rb):
                NT, r0 = tiles[rb], row_offs[rb]
                et_lo = et_pool.tile([128, KS // 2, RB], f8, tag="etl")
                et_hi = et_pool.tile([128, KS // 2, RB], f8, tag="eth")
                nc.sync.dma_start(
                    out=et_lo[:, :, :NT], in_=et_ap[:, : KS // 2, r0 : r0 + NT]
                )
                nc.sync.dma_start(
                    out=et_hi[:, :, :NT], in_=et_ap[:, KS // 2 :, r0 : r0 + NT]
                )
                et_tiles[rb] = (et_lo, et_hi)

            # --- rb 0 inputs: per-K-pair E^T tiles interleaved with the W
            # pair loads, so the first row block computes k-progressively
            # while W streams from HBM.
            NT0 = tiles[0]
            et0_p = []
            w_p = []
            for kp in range(KP):
                t = et0_pool.tile([128, 2, RB], f8, tag=f"et0_{kp}")
                wk = singles.tile([128, 2, D], f8, tag=f"w{kp}")
                nc.sync.dma_start(
                    out=t[:, :, :NT0], in_=et_ap[:, 2 * kp : 2 * kp + 2, 0:NT0]
                )
                nc.sync.dma_start(out=wk[:], in_=w_ap[:, 2 * kp : 2 * kp + 2, :])
                et0_p.append(t)
                w_p.append(wk)
            v_sb = singles.tile([128, MO], f32)
            nc.sync.dma_start(out=v_sb[:], in_=v_ap)
            bias_sb = singles.tile([128, MO, n_sub], f32)
            nc.sync.dma_start(out=bias_sb[:], in_=b_ap)

            for _rb in range(1, 7):
                if len(tiles) > _rb:
                    issue_et(_rb)

            def tanh_vdot(ps, m, NT, row0, acc_prev):
                """tanh(+bias) into fp16, then acc += v_m * th on the DVE.
                Returns the new accumulator tile."""
                th = tanh_pool.tile([128, RB], f16)
                for h in range(NT // SUB):
                    s_idx = row0 // SUB + h
                    nc.scalar.activation(
                        out=th[:, h * SUB : (h + 1) * SUB],
                        in_=ps[:, h * SUB : (h + 1) * SUB],
                        func=mybir.ActivationFunctionType.Tanh,
                        bias=bias_sb[:, m, s_idx : s_idx + 1],
                        scale=inv_scale,
                    )
                acc = acc_pool.tile([128, RB], f16)
                if acc_prev is None:
                    nc.vector.tensor_scalar_mul(
                        acc[:, :NT], th[:, :NT], v_sb[:, m : m + 1]
                    )
                else:
                    nc.vector.scalar_tensor_tensor(
                        out=acc[:, :NT],
                        in0=th[:, :NT],
                        scalar=v_sb[:, m : m + 1],
                        in1=acc_prev[:, :NT],
                        op0=mybir.AluOpType.mult,
                        op1=mybir.AluOpType.add,
                    )
                return acc

            def flush_scores(pend):
                """Partition-reduce a finished accumulator into [1, NT]
                scores and DMA it out.  Called one block late so the
                ones-matmul (which waits on the whole DVE chain) doesn't
                stall the PE queue ahead of the next block's matmuls."""
                acc, NT, row0 = pend
                sc_ps = psum_s.tile([1, RB], f32)
                nc.tensor.matmul(
                    sc_ps[:, :NT],
                    ones_sb[:],
                    acc[:, :NT],
                    start=True,
                    stop=True,
                )
                sc_sb = sc_pool.tile([1, RB], f32)
                # split the tiny score copies between ACT and DVE so
                # neither engine eats all 9 of them
                if (row0 // RB) % 2 == 0:
                    nc.vector.tensor_copy(sc_sb[:, :NT], sc_ps[:, :NT])
                else:
                    nc.scalar.copy(sc_sb[:, :NT], sc_ps[:, :NT])
                nc.sync.dma_start(
                    out=out_d[0:1, row0 : row0 + NT], in_=sc_sb[:, :NT]
                )

            pending = None
            for rb, NT in enumerate(tiles):
                row0 = row_offs[rb]
                if rb + 7 < len(tiles):
                    issue_et(rb + 7)
                acc = None
                if rb == 0:
                    # k-outer halves: 4 open PSUM banks accumulate while the
                    # (w_kp, et0_kp) pair chunks stream in.
                    ps_halves = []
                    for half in range(2):
                        ps_l = [
                            psum_e.tile(
                                [128, RB], f32, name=f"ps0_{half}_{mi}", tag="ps"
                            )
                            for mi in range(4)
                        ]
                        for kp in range(KP):
                            for mi in range(4):
                                m = half * 4 + mi
                                nc.tensor.matmul(
                                    ps_l[mi][:, :NT],
                                    w_p[kp][:, :, m * 128 : (m + 1) * 128],
                                    et0_p[kp][:, :, :NT],
                                    start=(kp == 0),
                                    stop=(kp == KP - 1),
                                    perf_mode=DR,
                                )
                        ps_halves.append(ps_l)
                    for half in range(2):
                        for mi in range(4):
                            acc = tanh_vdot(
                                ps_halves[half][mi], half * 4 + mi, NT, row0, acc
                            )
                else:
                    et_half = et_tiles.pop(rb)
                    for m in range(MO):
                        ps = psum_e.tile([128, RB], f32, tag="ps")
                        for kp in range(KP):
                            nc.tensor.matmul(
                                ps[:, :NT],
                                w_p[kp][:, :, m * 128 : (m + 1) * 128],
                                et_half[kp // 2][:, 2 * (kp % 2) : 2 * (kp % 2) + 2, :NT],
                                start=(kp == 0),
                                stop=(kp == KP - 1),
                                perf_mode=DR,
                            )
                        acc = tanh_vdot(ps, m, NT, row0, acc)
                if pending is not None:
                    flush_scores(pending)
                pending = (acc, NT, row0)
            flush_scores(pending)
    nc.compile()
    return nc


def _plan(src_len):
    """Pack valid rows (padded to SUB per batch) and balance across cores."""
    lens = np.clip(np.asarray(src_len).astype(np.int64), 1, S)
    units = (lens + SUB - 1) // SUB  # per-batch load in SUB units
    order = np.argsort(-units, kind="stable")
    loads = [0] * N_CORES
    core_batches = [[] for _ in range(N_CORES)]
    for b in order:
        c = min(range(N_CORES), key=lambda k: loads[k])
        loads[c] += int(units[b])
        core_batches[c].append(int(b))
    # local refinement: move/swap batches off the argmax core while it helps
    for _ in range(64):
        hi = max(range(N_CORES), key=lambda k: loads[k])
        best = None
        for bi, b in enumerate(core_batches[hi]):
            for c in range(N_CORES):
                if c == hi:
                    continue
                # move b to c
                new_max = max(
                    loads[c] + units[b],
                    *(loads[k] for k in range(N_CORES) if k not in (hi, c)),
                    loads[hi] - units[b],
                )
                if new_max < max(loads) and (best is None or new_max < best[0]):
                    best = (new_max, bi, c, None)
                for cj, b2 in enumerate(core_batches[c]):
                    d = units[b] - units[b2]
                    if d <= 0:
                        continue
                    new_max = max(
                        loads[c] + d,
                        *(loads[k] for k in range(N_CORES) if k not in (hi, c)),
                        loads[hi] - d,
                    )
                    if new_max < max(loads) and (best is None or new_max < best[0]):
                        best = (new_max, bi, c, cj)
        if best is None:
            break
        _, bi, c, cj = best
        b = core_batches[hi].pop(bi)
        if cj is None:
            core_batches[c].append(b)
            loads[hi] -= units[b]
            loads[c] += units[b]
        else:
            b2 = core_batches[c][cj]
            core_batches[c][cj] = b
            core_batches[hi].append(b2)
            loads[hi] += units[b2] - units[b]
            loads[c] += units[b] - units[b2]
    r_max = max(loads) * SUB
    R = ((r_max + 255) // 256) * 256
    # layout: per core, list of (batch, row_offset, valid_len, padded_len)
    layout = []
    for c in range(N_CORES):
        cur = 0
        segs = []
        for b in core_batches[c]:
            segs.append((b, cur, int(lens[b]), int(units[b]) * SUB))
            cur += int(units[b]) * SUB
        layout.append(segs)
    return R, layout


def _run(inputs, trace=False):
    if trace or os.environ.get("BASS_TRACE"):
        _ensure_trace_support()
    import ml_dtypes

    from concourse.bass_utils import run_bass_kernel_spmd

    f8 = ml_dtypes.float8_e4m3

    hidden = np.ascontiguousarray(np.asarray(inputs["hidden"]), dtype=np.float32)
    enc = np.asarray(inputs["encoder_outputs"])
    W = np.ascontiguousarray(np.asarray(inputs["W"]), dtype=np.float32)
    bvec = np.ascontiguousarray(np.asarray(inputs["b"]), dtype=np.float32)
    v = np.ascontiguousarray(np.asarray(inputs["v"]), dtype=np.float32)
    src_len = np.asarray(inputs["src_len"])

    # host-side: per-batch bias = hidden @ W[:D] + b   (0.4% of the FLOPs)
    bias_all = (hidden @ W[:D]).astype(np.float32) + bvec[None, :]  # [B, D]
    w8 = np.ascontiguousarray((W[D:] * W_SCALE).astype(f8))  # [D, D] fp8

    R, layout = _plan(src_len)
    n_sub = R // SUB

    in_maps = []
    for c in range(N_CORES):
        et = np.zeros((D, R), dtype=f8)
        bt = np.zeros((D, n_sub), dtype=np.float32)
        for b, off, ln, pad in layout[c]:
            eb = np.asarray(enc[b, :ln, :], dtype=np.float32) * E_SCALE
            et[:, off : off + ln] = eb.T.astype(f8)
            bt[:, off // SUB : (off + pad) // SUB] = bias_all[b][:, None]
        in_maps.append({"et": et, "wt": w8, "vt": v, "bt": bt})

    if R not in _NC_CACHE:
        _NC_CACHE[R] = _build_bass(R)
    nc = _NC_CACHE[R]

    res = run_bass_kernel_spmd(nc, in_maps, list(range(N_CORES)), trace=trace)

    attn = np.zeros((B, 1, S), dtype=np.float32)
    for c in range(N_CORES):
        sc = res.results[c]["scores"][0]
        for b, off, ln, _pad in layout[c]:
            srow = sc[off : off + ln].astype(np.float32)
            m = srow.max()
            e = np.exp(srow - m, dtype=np.float32)
            attn[b, 0, :ln] = e / e.sum(dtype=np.float32)
    return attn, res


def kernel(**inputs):
    attn, _ = _run(inputs, trace=False)
    return attn


# revision 16
# speedup vs baseline: 1.1552x; 1.1552x over previous
"""Bahdanau attention kernel for Trainium2 (8 NeuronCores) — fp8 DoubleRow.

Reference computation (B=32, S=2048, D=1024):
    x      = concat([broadcast(hidden), encoder_outputs], -1)   # [B,S,2D]
    energy = tanh(x @ W + b)                                    # [B,S,D]
    scores = energy . v                                         # [B,S]
    attn   = softmax(mask(scores, src_len))                     # [B,1,S]

Key transformations:
  * x @ W = encoder_outputs @ W[D:] + (hidden @ W[:D]); the hidden part is a
    tiny per-batch bias computed on the host and folded into the tanh bias.
  * rows with s >= src_len[b] are masked out of the softmax, so they are
    never computed: the host packs only the valid rows (padded to SUB=256
    per batch), load-balances batches across the 8 cores, and the device
    runs a dense kernel on the packed rows.
  * the big matmul runs in fp8(e4m3) with the PE's DoubleRow perf mode:
    256-deep contraction per pass, 2x bf16/fp32r throughput.  E is scaled
    by 16 and W by 256 on the host to center the e4m3 dynamic range; the
    exact 1/4096 rescale rides the tanh's activation scale.
  * the v-dot runs off the PE: ACT emits fp16 tanh tiles, the DVE
    accumulates v_m * th_m across the 8 D-chunks (2x fp16 mode), and a
    single fp16 ones-matmul reduces partitions into the [1, R] scores.
  * masking + softmax run on the host (cheap, O(B*S)).
"""

import os
import sys

import numpy as np

for _p in ("/root/.axon_site/_ro/trn_rl_repo", "/opt/trn_rl_repo"):
    if os.path.isdir(_p) and _p not in sys.path:
        sys.path.append(_p)

B, S, D = 32, 2048, 1024
N_CORES = 8
SUB = 256  # per-batch row padding == tanh bias subtile width
RB = 512  # main matmul row tile (PSUM free dim)
KS = D // 128  # 128-deep K chunks (8)
KP = KS // 2  # DoubleRow K pairs (4)
MO = D // 128  # D_out chunks (8)
E_SCALE = 16.0  # host scale on E before e4m3 quantization
W_SCALE = 256.0  # host scale on W before e4m3 quantization

_NC_CACHE = {}


def _ensure_trace_support():
    """Make trace=True / BASS_TRACE=1 runs survive on images where
    ``antenv.axon_hooks`` is absent (the boot shim degrades silently but
    ``bass_utils`` imports it unconditionally) and where artifact uploads
    to remote storage are unavailable.  No-ops when everything exists."""
    import types

    try:
        import antenv

        try:
            import antenv.axon_hooks  # noqa: F401
        except ImportError:
            mod = types.ModuleType("antenv.axon_hooks")
            state = {"hook": None}
            mod.set_axon_ntff_profile_hook = lambda h: state.__setitem__("hook", h)
            mod.get_axon_ntff_profile_hook = lambda: state["hook"]
            sys.modules["antenv.axon_hooks"] = mod
            antenv.axon_hooks = mod
            try:
                from trn_agent_boot.trn_boot import _ntff_profile_via_ctypes

                so = "/opt/axon/libaxon_pjrt.so"
                if os.path.exists(so):
                    mod.set_axon_ntff_profile_hook(_ntff_profile_via_ctypes(so))
            except Exception:
                pass
    except Exception:
        pass
    try:
        import concourse.bass_utils as bu

        orig = bu.upload_artifacts
        if not getattr(orig, "_safe_wrapped", False):

            def _safe_upload(tmpdir, _orig=orig):
                try:
                    return _orig(tmpdir)
                except Exception:
                    return f"local://{tmpdir}"

            _safe_upload._safe_wrapped = True
            bu.upload_artifacts = _safe_upload
    except Exception:
        pass


def _row_tiles(R):
    """Row-tile sizes covering R rows: 512-tiles plus an optional 256 tail."""
    tiles = [RB] * (R // RB)
    if R % RB:
        assert R % RB == 256
        tiles.append(256)
    return tiles


def _build_bass(R):
    """Build the per-core SPMD program for R packed rows (R % 256 == 0)."""
    import concourse.bass as bass  # noqa: F401
    import concourse.tile as tile
    from concourse import bacc, mybir

    f32 = mybir.dt.float32
    f16 = mybir.dt.float16
    f8 = mybir.dt.float8e4
    DR = mybir.MatmulPerfMode.DoubleRow
    n_sub = R // SUB
    tiles = _row_tiles(R)
    inv_scale = 1.0 / (E_SCALE * W_SCALE)

    nc = bacc.Bacc()
    et_d = nc.dram_tensor("et", [D, R], f8, kind="ExternalInput")
    w_d = nc.dram_tensor("wt", [D, D], f8, kind="ExternalInput")
    v_d = nc.dram_tensor("vt", [D], f32, kind="ExternalInput")
    b_d = nc.dram_tensor("bt", [D, n_sub], f32, kind="ExternalInput")
    out_d = nc.dram_tensor("scores", [1, R], f32, kind="ExternalOutput")

    et_ap = et_d[:, :].rearrange("(ks p) r -> p ks r", p=128)
    w_ap = w_d[:, :].rearrange("(ks p) j -> p ks j", p=128)
    v_ap = v_d[:].rearrange("(mo p) -> p mo", p=128)
    b_ap = b_d[:, :].rearrange("(mo p) s -> p mo s", p=128)

    with tile.TileContext(nc) as tc:
        with (
            tc.tile_pool(name="singles", bufs=1) as singles,
            tc.tile_pool(name="warm", bufs=1) as warm,
            tc.tile_pool(name="et0", bufs=1) as et0_pool,
            tc.tile_pool(name="et", bufs=12) as et_pool,
            tc.tile_pool(name="tanh", bufs=6) as tanh_pool,
            tc.tile_pool(name="acc", bufs=10) as acc_pool,
            tc.tile_pool(name="sc", bufs=2) as sc_pool,
            tc.tile_pool(name="psum_e", bufs=7, space="PSUM") as psum_e,
            tc.tile_pool(name="psum_s", bufs=1, space="PSUM") as psum_s,
        ):
            # --- rb 0 inputs first, on TWO DGE queues in parallel: W pairs
            # + v + bias ride the Activation engine's queue while the E^T
            # tiles ride Sync's, so the first matmul's 384KB critical load
            # halves in latency.  Issued before the ACT warmup so the tanh
            # table load doesn't delay the W transfers.
            NT0 = tiles[0]
            et0_p = []
            w_p = []
            for kp in range(KP):
                t = et0_pool.tile([128, 2, RB], f8, tag=f"et0_{kp}")
                wk = singles.tile([128, 2, D], f8, tag=f"w{kp}")
                nc.sync.dma_start(
                    out=t[:, :, :NT0], in_=et_ap[:, 2 * kp : 2 * kp + 2, 0:NT0]
                )
                nc.scalar.dma_start(out=wk[:], in_=w_ap[:, 2 * kp : 2 * kp + 2, :])
                et0_p.append(t)
                w_p.append(wk)
            v_sb = singles.tile([128, MO], f32)
            nc.scalar.dma_start(out=v_sb[:], in_=v_ap)
            bias_sb = singles.tile([128, MO, n_sub], f32)
            nc.scalar.dma_start(out=bias_sb[:], in_=b_ap)

            # --- warmup: load the ACT tanh table and spin the PE on junk
            # fp8 matmuls while the first real DMAs are in flight (HAM
            # un-throttles after ~3.4us of PE activity; the ACT table load
            # costs ~1.5us once).
            wact = warm.tile([128, 2], f32)
            nc.vector.memset(wact[:], 0.0)
            nc.scalar.activation(
                out=wact[:, 1:2],
                in_=wact[:, 0:1],
                func=mybir.ActivationFunctionType.Tanh,
                bias=0.0,
                scale=1.0,
            )
            ones_sb = singles.tile([128, 1], f16)
            nc.vector.memset(ones_sb[:], 1.0)
            wf8 = warm.tile([128, 2, 64], f8)
            nc.vector.memset(wf8[:], 0.0)
            ps_w = psum_e.tile([64, 64], f32, name="ps_warm", tag="ps")
            for _ in range(16):
                nc.tensor.matmul(
                    ps_w[:], wf8[:], wf8[:, :, :], start=True, stop=True,
                    perf_mode=DR,
                )

            # --- E^T row-block loads: two half-K DMAs per block so
            # dependencies unblock earlier.  Issued with a prefetch depth
            # of 6 blocks; block 0 is loaded per-K-pair (below) so the
            # pipeline can start as early as possible.
            row_offs = []
            r0 = 0
            for NT in tiles:
                row_offs.append(r0)
                r0 += NT

            et_tiles = {}

            def issue_et(rb):
                NT, r0 = tiles[rb], row_offs[rb]
                et_lo = et_pool.tile([128, KS // 2, RB], f8, tag="etl")
                et_hi = et_pool.tile([128, KS // 2, RB], f8, tag="eth")
                nc.sync.dma_start(
                    out=et_lo[:, :, :NT], in_=et_ap[:, : KS // 2, r0 : r0 + NT]
                )
                nc.sync.dma_start(
                    out=et_hi[:, :, :NT], in_=et_ap[:, KS // 2 :, r0 : r0 + NT]
                )
                et_tiles[rb] = (et_lo, et_hi)

            for _rb in range(1, 7):
                if len(tiles) > _rb:
                    issue_et(_rb)

            def tanh_vdot(ps, m, NT, row0, acc_prev):
                """tanh(+bias) into fp16, then acc += v_m * th on the DVE.
                Returns the new accumulator tile."""
                th = tanh_pool.tile([128, RB], f16)
                for h in range(NT // SUB):
                    s_idx = row0 // SUB + h
                    nc.scalar.activation(
                        out=th[:, h * SUB : (h + 1) * SUB],
                        in_=ps[:, h * SUB : (h + 1) * SUB],
                        func=mybir.ActivationFunctionType.Tanh,
                        bias=bias_sb[:, m, s_idx : s_idx + 1],
                        scale=inv_scale,
                    )
                acc = acc_pool.tile([128, RB], f16)
                if acc_prev is None:
                    nc.vector.tensor_scalar_mul(
                        acc[:, :NT], th[:, :NT], v_sb[:, m : m + 1]
                    )
                else:
                    nc.vector.scalar_tensor_tensor(
                        out=acc[:, :NT],
                        in0=th[:, :NT],
                        scalar=v_sb[:, m : m + 1],
                        in1=acc_prev[:, :NT],
                        op0=mybir.AluOpType.mult,
                        op1=mybir.AluOpType.add,
                    )
                return acc

            def flush_scores(pend):
                """Partition-reduce a finished accumulator into [1, NT]
                scores and DMA it out.  Called one block late so the
                ones-matmul (which waits on the whole DVE chain) doesn't
                stall the PE queue ahead of the next block's matmuls."""
                acc, NT, row0 = pend
                sc_ps = psum_s.tile([1, RB], f32)
                nc.tensor.matmul(
                    sc_ps[:, :NT],
                    ones_sb[:],
                    acc[:, :NT],
                    start=True,
                    stop=True,
                )
                sc_sb = sc_pool.tile([1, RB], f32)
                # split the tiny score copies between ACT and DVE so
                # neither engine eats all 9 of them
                if (row0 // RB) % 2 == 0:
                    nc.vector.tensor_copy(sc_sb[:, :NT], sc_ps[:, :NT])
                else:
                    nc.scalar.copy(sc_sb[:, :NT], sc_ps[:, :NT])
                nc.sync.dma_start(
                    out=out_d[0:1, row0 : row0 + NT], in_=sc_sb[:, :NT]
                )

            pending = None
            for rb, NT in enumerate(tiles):
                row0 = row_offs[rb]
                if rb + 7 < len(tiles):
                    issue_et(rb + 7)
                acc = None
                if rb == 0:
                    # k-outer halves: 4 open PSUM banks accumulate while the
                    # (w_kp, et0_kp) pair chunks stream in.
                    ps_halves = []
                    for half in range(2):
                        ps_l = [
                            psum_e.tile(
                                [128, RB], f32, name=f"ps0_{half}_{mi}", tag="ps"
                            )
                            for mi in range(4)
                        ]
                        for kp in range(KP):
                            for mi in range(4):
                                m = half * 4 + mi
                                nc.tensor.matmul(
                                    ps_l[mi][:, :NT],
                                    w_p[kp][:, :, m * 128 : (m + 1) * 128],
                                    et0_p[kp][:, :, :NT],
                                    start=(kp == 0),
                                    stop=(kp == KP - 1),
                                    perf_mode=DR,
                                )
                        ps_halves.append(ps_l)
                    for half in range(2):
                        for mi in range(4):
                            acc = tanh_vdot(
                                ps_halves[half][mi], half * 4 + mi, NT, row0, acc
                            )
                else:
                    et_half = et_tiles.pop(rb)
                    for m in range(MO):
                        ps = psum_e.tile([128, RB], f32, tag="ps")
                        for kp in range(KP):
                            nc.tensor.matmul(
                                ps[:, :NT],
                                w_p[kp][:, :, m * 128 : (m + 1) * 128],
                                et_half[kp // 2][:, 2 * (kp % 2) : 2 * (kp % 2) + 2, :NT],
                                start=(kp == 0),
                                stop=(kp == KP - 1),
                                perf_mode=DR,
                            )
                        acc = tanh_vdot(ps, m, NT, row0, acc)
                if pending is not None:
                    flush_scores(pending)
                pending = (acc, NT, row0)
            flush_scores(pending)
    nc.compile()
    return nc


def _plan(src_len):
    """Pack valid rows (padded to SUB per batch) and balance across cores."""
    lens = np.clip(np.asarray(src_len).astype(np.int64), 1, S)
    units = (lens + SUB - 1) // SUB  # per-batch load in SUB units
    order = np.argsort(-units, kind="stable")
    loads = [0] * N_CORES
    core_batches = [[] for _ in range(N_CORES)]
    for b in order:
        c = min(range(N_CORES), key=lambda k: loads[k])
        loads[c] += int(units[b])
        core_batches[c].append(int(b))
    # local refinement: move/swap batches off the argmax core while it helps
    for _ in range(64):
        hi = max(range(N_CORES), key=lambda k: loads[k])
        best = None
        for bi, b in enumerate(core_batches[hi]):
            for c in range(N_CORES):
                if c == hi:
                    continue
                # move b to c
                new_max = max(
                    loads[c] + units[b],
                    *(loads[k] for k in range(N_CORES) if k not in (hi, c)),
                    loads[hi] - units[b],
                )
                if new_max < max(loads) and (best is None or new_max < best[0]):
                    best = (new_max, bi, c, None)
                for cj, b2 in enumerate(core_batches[c]):
                    d = units[b] - units[b2]
                    if d <= 0:
                        continue
                    new_max = max(
                        loads[c] + d,
                        *(loads[k] for k in range(N_CORES) if k not in (hi, c)),
                        loads[hi] - d,
                    )
                    if new_max < max(loads) and (best is None or new_max < best[0]):
                        best = (new_max, bi, c, cj)
        if best is None:
            break
        _, bi, c, cj = best
        b = core_batches[hi].pop(bi)
        if cj is None:
            core_batches[c].append(b)
            loads[hi] -= units[b]
            loads[c] += units[b]
        else:
            b2 = core_batches[c][cj]
            core_batches[c][cj] = b
            core_batches[hi].append(b2)
            loads[hi] += units[b2] - units[b]
            loads[c] += units[b] - units[b2]
    r_max = max(loads) * SUB
    R = ((r_max + 255) // 256) * 256
    # layout: per core, list of (batch, row_offset, valid_len, padded_len)
    layout = []
    for c in range(N_CORES):
        cur = 0
        segs = []
        for b in core_batches[c]:
            segs.append((b, cur, int(lens[b]), int(units[b]) * SUB))
            cur += int(units[b]) * SUB
        layout.append(segs)
    return R, layout


def _run(inputs, trace=False):
    if trace or os.environ.get("BASS_TRACE"):
        _ensure_trace_support()
    import ml_dtypes

    from concourse.bass_utils import run_bass_kernel_spmd

    f8 = ml_dtypes.float8_e4m3

    hidden = np.ascontiguousarray(np.asarray(inputs["hidden"]), dtype=np.float32)
    enc = np.asarray(inputs["encoder_outputs"])
    W = np.ascontiguousarray(np.asarray(inputs["W"]), dtype=np.float32)
    bvec = np.ascontiguousarray(np.asarray(inputs["b"]), dtype=np.float32)
    v = np.ascontiguousarray(np.asarray(inputs["v"]), dtype=np.float32)
    src_len = np.asarray(inputs["src_len"])

    # host-side: per-batch bias = hidden @ W[:D] + b   (0.4% of the FLOPs)
    bias_all = (hidden @ W[:D]).astype(np.float32) + bvec[None, :]  # [B, D]
    w8 = np.ascontiguousarray((W[D:] * W_SCALE).astype(f8))  # [D, D] fp8

    R, layout = _plan(src_len)
    n_sub = R // SUB

    in_maps = []
    for c in range(N_CORES):
        et = np.zeros((D, R), dtype=f8)
        bt = np.zeros((D, n_sub), dtype=np.float32)
        for b, off, ln, pad in layout[c]:
            eb = np.asarray(enc[b, :ln, :], dtype=np.float32) * E_SCALE
            et[:, off : off + ln] = eb.T.astype(f8)
            bt[:, off // SUB : (off + pad) // SUB] = bias_all[b][:, None]
        in_maps.append({"et": et, "wt": w8, "vt": v, "bt": bt})

    if R not in _NC_CACHE:
        _NC_CACHE[R] = _build_bass(R)
    nc = _NC_CACHE[R]

    res = run_bass_kernel_spmd(nc, in_maps, list(range(N_CORES)), trace=trace)

    attn = np.zeros((B, 1, S), dtype=np.float32)
    for c in range(N_CORES):
        sc = res.results[c]["scores"][0]
        for b, off, ln, _pad in layout[c]:
            srow = sc[off : off + ln].astype(np.float32)
            m = srow.max()
            e = np.exp(srow - m, dtype=np.float32)
            attn[b, 0, :ln] = e / e.sum(dtype=np.float32)
    return attn, res


def kernel(**inputs):
    attn, _ = _run(inputs, trace=False)
    return attn


# revision 22
# speedup vs baseline: 1.2115x; 1.0488x over previous
"""Bahdanau attention kernel for Trainium2 (8 NeuronCores) — fp8 DoubleRow.

Reference computation (B=32, S=2048, D=1024):
    x      = concat([broadcast(hidden), encoder_outputs], -1)   # [B,S,2D]
    energy = tanh(x @ W + b)                                    # [B,S,D]
    scores = energy . v                                         # [B,S]
    attn   = softmax(mask(scores, src_len))                     # [B,1,S]

Key transformations:
  * x @ W = encoder_outputs @ W[D:] + (hidden @ W[:D]); the hidden part is a
    tiny per-batch bias computed on the host and folded into the tanh bias.
  * rows with s >= src_len[b] are masked out of the softmax, so they are
    never computed: the host packs only the valid rows (padded to SUB=256
    per batch), load-balances batches across the 8 cores, and the device
    runs a dense kernel on the packed rows.
  * the big matmul runs in fp8(e4m3) with the PE's DoubleRow perf mode:
    256-deep contraction per pass, 2x bf16/fp32r throughput.  E is scaled
    by 16 and W by 256 on the host to center the e4m3 dynamic range; the
    exact 1/4096 rescale rides the tanh's activation scale.
  * the v-dot runs off the PE: ACT emits fp16 tanh tiles, the DVE
    accumulates v_m * th_m across the 8 D-chunks (2x fp16 mode), and a
    single fp16 ones-matmul reduces partitions into the [1, R] scores.
  * masking + softmax run on the host (cheap, O(B*S)).
"""

import os
import sys

import numpy as np

for _p in ("/root/.axon_site/_ro/trn_rl_repo", "/opt/trn_rl_repo"):
    if os.path.isdir(_p) and _p not in sys.path:
        sys.path.append(_p)

B, S, D = 32, 2048, 1024
N_CORES = 8
SUB = 256  # per-batch row padding == tanh bias subtile width
RB = 512  # main matmul row tile (PSUM free dim)
KS = D // 128  # 128-deep K chunks (8)
KP = KS // 2  # DoubleRow K pairs (4)
MO = D // 128  # D_out chunks (8)
E_SCALE = 16.0  # host scale on E before e4m3 quantization
W_SCALE = 256.0  # host scale on W before e4m3 quantization

_NC_CACHE = {}


def _ensure_trace_support():
    """Make trace=True / BASS_TRACE=1 runs survive on images where
    ``antenv.axon_hooks`` is absent (the boot shim degrades silently but
    ``bass_utils`` imports it unconditionally) and where artifact uploads
    to remote storage are unavailable.  No-ops when everything exists."""
    import types

    try:
        import antenv

        try:
            import antenv.axon_hooks  # noqa: F401
        except ImportError:
            mod = types.ModuleType("antenv.axon_hooks")
            state = {"hook": None}
            mod.set_axon_ntff_profile_hook = lambda h: state.__setitem__("hook", h)
            mod.get_axon_ntff_profile_hook = lambda: state["hook"]
            sys.modules["antenv.axon_hooks"] = mod
            antenv.axon_hooks = mod
            try:
                from trn_agent_boot.trn_boot import _ntff_profile_via_ctypes

                so = "/opt/axon/libaxon_pjrt.so"
                if os.path.exists(so):
                    mod.set_axon_ntff_profile_hook(_ntff_profile_via_ctypes(so))
            except Exception:
                pass
    except Exception:
        pass
    try:
        import concourse.bass_utils as bu

        orig = bu.upload_artifacts
        if not getattr(orig, "_safe_wrapped", False):

            def _safe_upload(tmpdir, _orig=orig):
                try:
                    return _orig(tmpdir)
                except Exception:
                    return f"local://{tmpdir}"

            _safe_upload._safe_wrapped = True
            bu.upload_artifacts = _safe_upload
    except Exception:
        pass


def _row_tiles(R):
    """Row-tile sizes covering R rows: 512-tiles plus an optional 256 tail."""
    tiles = [RB] * (R // RB)
    if R % RB:
        assert R % RB == 256
        tiles.append(256)
    return tiles


def _build_bass(R):
    """Build the per-core SPMD program for R packed rows (R % 256 == 0)."""
    import concourse.bass as bass  # noqa: F401
    import concourse.tile as tile
    from concourse import bacc, mybir

    f32 = mybir.dt.float32
    f16 = mybir.dt.float16
    f8 = mybir.dt.float8e4
    DR = mybir.MatmulPerfMode.DoubleRow
    n_sub = R // SUB
    tiles = _row_tiles(R)
    inv_scale = 1.0 / (E_SCALE * W_SCALE)

    nc = bacc.Bacc()
    et_d = nc.dram_tensor("et", [D, R], f8, kind="ExternalInput")
    w_d = nc.dram_tensor("wt", [D, D], f8, kind="ExternalInput")
    v_d = nc.dram_tensor("vt", [D], f32, kind="ExternalInput")
    b_d = nc.dram_tensor("bt", [D, n_sub], f32, kind="ExternalInput")
    out_d = nc.dram_tensor("scores", [1, R], f32, kind="ExternalOutput")

    et_ap = et_d[:, :].rearrange("(ks p) r -> p ks r", p=128)
    w_ap = w_d[:, :].rearrange("(ks p) j -> p ks j", p=128)
    v_ap = v_d[:].rearrange("(mo p) -> p mo", p=128)
    b_ap = b_d[:, :].rearrange("(mo p) s -> p mo s", p=128)

    with tile.TileContext(nc) as tc:
        with (
            tc.tile_pool(name="singles", bufs=1) as singles,
            tc.tile_pool(name="warm", bufs=1) as warm,
            tc.tile_pool(name="et0", bufs=1) as et0_pool,
            tc.tile_pool(name="et", bufs=12) as et_pool,
            tc.tile_pool(name="tanh", bufs=8) as tanh_pool,
            tc.tile_pool(name="acc", bufs=12) as acc_pool,
            tc.tile_pool(name="sc", bufs=2) as sc_pool,
            tc.tile_pool(name="psum_e", bufs=7, space="PSUM") as psum_e,
            tc.tile_pool(name="psum_s", bufs=1, space="PSUM") as psum_s,
        ):
            # --- rb 0 inputs first, on TWO DGE queues in parallel: W pairs
            # + v + bias ride the Activation engine's queue while the E^T
            # tiles ride Sync's, so the first matmul's 384KB critical load
            # halves in latency.  Issued before the ACT warmup so the tanh
            # table load doesn't delay the W transfers.
            NT0 = tiles[0]
            et0_p = []
            w_p = []
            for kp in range(KP):
                t = et0_pool.tile([128, 2, RB], f8, tag=f"et0_{kp}")
                wk = singles.tile([128, 2, D], f8, tag=f"w{kp}")
                nc.sync.dma_start(
                    out=t[:, :, :NT0], in_=et_ap[:, 2 * kp : 2 * kp + 2, 0:NT0]
                )
                nc.scalar.dma_start(out=wk[:], in_=w_ap[:, 2 * kp : 2 * kp + 2, :])
                et0_p.append(t)
                w_p.append(wk)
            v_sb = singles.tile([128, MO], f32)
            nc.scalar.dma_start(out=v_sb[:], in_=v_ap)
            bias_sb = singles.tile([128, MO, n_sub], f32)
            nc.scalar.dma_start(out=bias_sb[:], in_=b_ap)

            # --- warmup: load the ACT tanh table and spin the PE on junk
            # fp8 matmuls while the first real DMAs are in flight (HAM
            # un-throttles after ~3.4us of PE activity; the ACT table load
            # costs ~1.5us once).
            wact = warm.tile([128, 2], f32)
            nc.vector.memset(wact[:], 0.0)
            nc.scalar.activation(
                out=wact[:, 1:2],
                in_=wact[:, 0:1],
                func=mybir.ActivationFunctionType.Tanh,
                bias=0.0,
                scale=1.0,
            )
            ones_sb = singles.tile([128, 1], f16)
            nc.vector.memset(ones_sb[:], 1.0)
            wf8 = warm.tile([128, 2, 64], f8)
            nc.vector.memset(wf8[:], 0.0)
            ps_w = psum_e.tile([64, 64], f32, name="ps_warm", tag="ps")
            for _ in range(16):
                nc.tensor.matmul(
                    ps_w[:], wf8[:], wf8[:, :, :], start=True, stop=True,
                    perf_mode=DR,
                )

            # --- E^T row-block loads: two half-K DMAs per block so
            # dependencies unblock earlier.  Issued with a prefetch depth
            # of 6 blocks; block 0 is loaded per-K-pair (below) so the
            # pipeline can start as early as possible.
            row_offs = []
            r0 = 0
            for NT in tiles:
                row_offs.append(r0)
                r0 += NT

            et_tiles = {}

            def issue_et(rb):
                NT, r0 = tiles[rb], row_offs[rb]
                et_lo = et_pool.tile([128, KS // 2, RB], f8, tag="etl")
                et_hi = et_pool.tile([128, KS // 2, RB], f8, tag="eth")
                nc.sync.dma_start(
                    out=et_lo[:, :, :NT], in_=et_ap[:, : KS // 2, r0 : r0 + NT]
                )
                nc.sync.dma_start(
                    out=et_hi[:, :, :NT], in_=et_ap[:, KS // 2 :, r0 : r0 + NT]
                )
                et_tiles[rb] = (et_lo, et_hi)

            for _rb in range(1, 6):
                if len(tiles) > _rb:
                    issue_et(_rb)

            def tanh_vdot(ps, m, NT, row0, acc_prev):
                """tanh(+bias) into fp16, then acc += v_m * th on the DVE.
                Returns the new accumulator tile.  Callers keep two
                independent chains (even/odd m) so the per-block DVE
                dependency chain is 4 deep instead of 8."""
                th = tanh_pool.tile([128, RB], f16)
                for h in range(NT // SUB):
                    s_idx = row0 // SUB + h
                    nc.scalar.activation(
                        out=th[:, h * SUB : (h + 1) * SUB],
                        in_=ps[:, h * SUB : (h + 1) * SUB],
                        func=mybir.ActivationFunctionType.Tanh,
                        bias=bias_sb[:, m, s_idx : s_idx + 1],
                        scale=inv_scale,
                    )
                acc = acc_pool.tile([128, RB], f16)
                if acc_prev is None:
                    nc.vector.tensor_scalar_mul(
                        acc[:, :NT], th[:, :NT], v_sb[:, m : m + 1]
                    )
                else:
                    nc.vector.scalar_tensor_tensor(
                        out=acc[:, :NT],
                        in0=th[:, :NT],
                        scalar=v_sb[:, m : m + 1],
                        in1=acc_prev[:, :NT],
                        op0=mybir.AluOpType.mult,
                        op1=mybir.AluOpType.add,
                    )
                return acc

            def merge_chains(acc_even, acc_odd, NT):
                acc = acc_pool.tile([128, RB], f16)
                nc.vector.tensor_tensor(
                    out=acc[:, :NT],
                    in0=acc_even[:, :NT],
                    in1=acc_odd[:, :NT],
                    op=mybir.AluOpType.add,
                )
                return acc

            def flush_scores(pend):
                """Partition-reduce a finished accumulator into [1, NT]
                scores and DMA it out.  Called one block late so the
                ones-matmul (which waits on the whole DVE chain) doesn't
                stall the PE queue ahead of the next block's matmuls."""
                acc, NT, row0 = pend
                sc_ps = psum_s.tile([1, RB], f32)
                nc.tensor.matmul(
                    sc_ps[:, :NT],
                    ones_sb[:],
                    acc[:, :NT],
                    start=True,
                    stop=True,
                )
                sc_sb = sc_pool.tile([1, RB], f32)
                # split the tiny score copies between ACT and DVE so
                # neither engine eats all 9 of them
                if (row0 // RB) % 2 == 0:
                    nc.vector.tensor_copy(sc_sb[:, :NT], sc_ps[:, :NT])
                else:
                    nc.scalar.copy(sc_sb[:, :NT], sc_ps[:, :NT])
                nc.sync.dma_start(
                    out=out_d[0:1, row0 : row0 + NT], in_=sc_sb[:, :NT]
                )

            pending = None
            for rb, NT in enumerate(tiles):
                row0 = row_offs[rb]
                if rb + 5 < len(tiles):
                    issue_et(rb + 5)
                chains = [None, None]
                if rb == 0:
                    # k-outer halves: 4 open PSUM banks accumulate while the
                    # (w_kp, et0_kp) pair chunks stream in.
                    ps_halves = []
                    for half in range(2):
                        ps_l = [
                            psum_e.tile(
                                [128, RB], f32, name=f"ps0_{half}_{mi}", tag="ps"
                            )
                            for mi in range(4)
                        ]
                        for kp in range(KP):
                            for mi in range(4):
                                m = half * 4 + mi
                                nc.tensor.matmul(
                                    ps_l[mi][:, :NT],
                                    w_p[kp][:, :, m * 128 : (m + 1) * 128],
                                    et0_p[kp][:, :, :NT],
                                    start=(kp == 0),
                                    stop=(kp == KP - 1),
                                    perf_mode=DR,
                                )
                        ps_halves.append(ps_l)
                    for half in range(2):
                        for mi in range(4):
                            m = half * 4 + mi
                            chains[m % 2] = tanh_vdot(
                                ps_halves[half][mi], m, NT, row0, chains[m % 2]
                            )
                else:
                    et_half = et_tiles.pop(rb)
                    for m in range(MO):
                        ps = psum_e.tile([128, RB], f32, tag="ps")
                        for kp in range(KP):
                            nc.tensor.matmul(
                                ps[:, :NT],
                                w_p[kp][:, :, m * 128 : (m + 1) * 128],
                                et_half[kp // 2][:, 2 * (kp % 2) : 2 * (kp % 2) + 2, :NT],
                                start=(kp == 0),
                                stop=(kp == KP - 1),
                                perf_mode=DR,
                            )
                        chains[m % 2] = tanh_vdot(ps, m, NT, row0, chains[m % 2])
                acc = merge_chains(chains[0], chains[1], NT)
                if pending is not None:
                    flush_scores(pending)
                pending = (acc, NT, row0)
            flush_scores(pending)
    nc.compile()
    return nc


def _plan(src_len):
    """Pack valid rows (padded to SUB per batch) and balance across cores."""
    lens = np.clip(np.asarray(src_len).astype(np.int64), 1, S)
    units = (lens + SUB - 1) // SUB  # per-batch load in SUB units
    order = np.argsort(-units, kind="stable")
    loads = [0] * N_CORES
    core_batches = [[] for _ in range(N_CORES)]
    for b in order:
        c = min(range(N_CORES), key=lambda k: loads[k])
        loads[c] += int(units[b])
        core_batches[c].append(int(b))
    # local refinement: move/swap batches off the argmax core while it helps
    for _ in range(64):
        hi = max(range(N_CORES), key=lambda k: loads[k])
        best = None
        for bi, b in enumerate(core_batches[hi]):
            for c in range(N_CORES):
                if c == hi:
                    continue
                # move b to c
                new_max = max(
                    loads[c] + units[b],
                    *(loads[k] for k in range(N_CORES) if k not in (hi, c)),
                    loads[hi] - units[b],
                )
                if new_max < max(loads) and (best is None or new_max < best[0]):
                    best = (new_max, bi, c, None)
                for cj, b2 in enumerate(core_batches[c]):
                    d = units[b] - units[b2]
                    if d <= 0:
                        continue
                    new_max = max(
                        loads[c] + d,
                        *(loads[k] for k in range(N_CORES) if k not in (hi, c)),
                        loads[hi] - d,
                    )
                    if new_max < max(loads) and (best is None or new_max < best[0]):
                        best = (new_max, bi, c, cj)
        if best is None:
            break
        _, bi, c, cj = best
        b = core_batches[hi].pop(bi)
        if cj is None:
            core_batches[c].append(b)
            loads[hi] -= units[b]
            loads[c] += units[b]
        else:
            b2 = core_batches[c][cj]
            core_batches[c][cj] = b
            core_batches[hi].append(b2)
            loads[hi] += units[b2] - units[b]
            loads[c] += units[b] - units[b2]
    r_max = max(loads) * SUB
    R = ((r_max + 255) // 256) * 256
    # layout: per core, list of (batch, row_offset, valid_len, padded_len)
    layout = []
    for c in range(N_CORES):
        cur = 0
        segs = []
        for b in core_batches[c]:
            segs.append((b, cur, int(lens[b]), int(units[b]) * SUB))
            cur += int(units[b]) * SUB
        layout.append(segs)
    return R, layout


def _run(inputs, trace=False):
    if trace or os.environ.get("BASS_TRACE"):
        _ensure_trace_support()
    import ml_dtypes

    from concourse.bass_utils import run_bass_kernel_spmd

    f8 = ml_dtypes.float8_e4m3

    hidden = np.ascontiguousarray(np.asarray(inputs["hidden"]), dtype=np.float32)
    enc = np.asarray(inputs["encoder_outputs"])
    W = np.ascontiguousarray(np.asarray(inputs["W"]), dtype=np.float32)
    bvec = np.ascontiguousarray(np.asarray(inputs["b"]), dtype=np.float32)
    v = np.ascontiguousarray(np.asarray(inputs["v"]), dtype=np.float32)
    src_len = np.asarray(inputs["src_len"])

    # host-side: per-batch bias = hidden @ W[:D] + b   (0.4% of the FLOPs)
    bias_all = (hidden @ W[:D]).astype(np.float32) + bvec[None, :]  # [B, D]
    w8 = np.ascontiguousarray((W[D:] * W_SCALE).astype(f8))  # [D, D] fp8

    R, layout = _plan(src_len)
    n_sub = R // SUB

    in_maps = []
    for c in range(N_CORES):
        et = np.zeros((D, R), dtype=f8)
        bt = np.zeros((D, n_sub), dtype=np.float32)
        for b, off, ln, pad in layout[c]:
            eb = np.asarray(enc[b, :ln, :], dtype=np.float32) * E_SCALE
            et[:, off : off + ln] = eb.T.astype(f8)
            bt[:, off // SUB : (off + pad) // SUB] = bias_all[b][:, None]
        in_maps.append({"et": et, "wt": w8, "vt": v, "bt": bt})

    if R not in _NC_CACHE:
        _NC_CACHE[R] = _build_bass(R)
    nc = _NC_CACHE[R]

    res = run_bass_kernel_spmd(nc, in_maps, list(range(N_CORES)), trace=trace)

    attn = np.zeros((B, 1, S), dtype=np.float32)
    for c in range(N_CORES):
        sc = res.results[c]["scores"][0]
        for b, off, ln, _pad in layout[c]:
            srow = sc[off : off + ln].astype(np.float32)
            m = srow.max()
            e = np.exp(srow - m, dtype=np.float32)
            attn[b, 0, :ln] = e / e.sum(dtype=np.float32)
    return attn, res


def kernel(**inputs):
    attn, _ = _run(inputs, trace=False)
    return attn
